# revision 19
# baseline (speedup 1.0000x reference)
"""Chamfer boundary SDF loss — Trainium2 Bass kernel (8 NeuronCores).

Strategy
--------
The reference output is a sum of f32 values interpolated exactly at SDF zero
crossings — analytically ~0, so the returned scalar is dominated by f32
rounding detail. Passing a relative-error gate therefore requires replicating
the reference's f32 arithmetic bit-exactly on the same backend (neuron/XLA
eager), not just approximating the math.

Decomposition:
  * All O(H*W) "cheap" ops (zero-crossing extraction, normals, bilinear
    weights, final gathers/scatter/sums) run as the *identical* eager jnp op
    sequence the reference executes (vmapped, B=1) — bit-identical by
    construction.
  * The O(M*N) nearest-neighbor search (M=N=18240 candidates, the dominant
    cost) runs on the 8 NeuronCores as a Bass SPMD kernel, sharded over the
    pred-candidate dim (143 blocks of 128). Each block only searches the gt
    candidates within +-4 grid rows (a "stripe" of <=2048 of the 18240
    candidates): any pred point whose true NN is farther than DIST_THRESHOLD=3
    contributes 0, and the stripe provably contains every global minimizer
    (and all argmin ties) whenever the threshold test can pass.
  * d2 bits match the reference exactly: the PE computes fl(2*cross) via a
    K=2 f32 matmul with pre-doubled pred coords (doubling commutes with
    round-to-nearest), and fl(sp+sg) via a second K=2 matmul against a ones
    row (a single-rounded fma of an exact product is an IEEE add). The DVE
    subtracts and clamps; max8/max_index implement first-occurrence argmin
    (on negated values) exactly like XLA's argmin combiner.
"""
import numpy as np

H = W = 96
NV = (H - 1) * W          # 9120 vertical-pair candidates
NH = H * (W - 1)          # 9120 horizontal-pair candidates
NP = NV + NH              # 18240
BLK = 128
NCORES = 8
PATCH = 11                # pred candidates per block = an 11x11 grid patch
STRIPE = 768              # gt candidates searched per block (patch +-4 window)
UPDATE_SCALE = 1.0
DIST_THRESHOLD = 3.0
W_INJECT = 1.0
W_PIXEL = 1.0
EPS = 1e-8
BIG = 1e6

_BASS_CACHE = {}
_PROFILE = False          # set True (e.g. by test.py) to capture a neuron profile
_LAST_EXEC_NS = None      # exec_time_ns of the last profiled run
_LAST_TRACE = None        # BassKernelResults of the last profiled run


# ---------------------------------------------------------------------------
# Eager jnp pieces — identical op sequences to the reference (vmapped, B=1)
# ---------------------------------------------------------------------------

def _jnp_funcs():
    import jax
    import jax.numpy as jnp

    def extract_zc(sdf):
        Hh, Ww = sdf.shape
        ii = jnp.arange(Hh, dtype=sdf.dtype)
        jj = jnp.arange(Ww, dtype=sdf.dtype)
        v1, v2 = sdf[:-1, :], sdf[1:, :]
        a = jnp.abs(v1) / (jnp.abs(v1) + jnp.abs(v2) + EPS)
        i0 = ii[:-1, None]
        rv = jnp.where(v1 == 0, i0, jnp.where(v2 == 0, i0 + 1.0, i0 + a))
        cv = jnp.broadcast_to(jj[None, :], v1.shape)
        mv = (v1 == 0) | (v2 == 0) | (v1 * v2 < 0)
        h1, h2 = sdf[:, :-1], sdf[:, 1:]
        b = jnp.abs(h1) / (jnp.abs(h1) + jnp.abs(h2) + EPS)
        j0 = jj[None, :-1]
        ch = jnp.where(h1 == 0, j0, jnp.where(h2 == 0, j0 + 1.0, j0 + b))
        rh = jnp.broadcast_to(ii[:, None], h1.shape)
        mh = (h1 == 0) | (h2 == 0) | (h1 * h2 < 0)
        pts = jnp.concatenate([
            jnp.stack([rv.ravel(), cv.ravel()], axis=1),
            jnp.stack([rh.ravel(), ch.ravel()], axis=1)], axis=0)
        mask = jnp.concatenate([mv.ravel(), mh.ravel()], axis=0)
        return pts, mask

    def compute_normals(sdf):
        gr = jnp.concatenate([sdf[1:2] - sdf[0:1], (sdf[2:] - sdf[:-2]) / 2.0,
                              sdf[-1:] - sdf[-2:-1]], axis=0)
        gc = jnp.concatenate([sdf[:, 1:2] - sdf[:, 0:1],
                              (sdf[:, 2:] - sdf[:, :-2]) / 2.0,
                              sdf[:, -1:] - sdf[:, -2:-1]], axis=1)
        return jnp.stack([gr, gc], axis=2)

    def _corners(pts, Hh, Ww):
        r, c = pts[:, 0], pts[:, 1]
        r0 = jnp.clip(jnp.floor(r).astype(jnp.int32), 0, Hh - 1)
        c0 = jnp.clip(jnp.floor(c).astype(jnp.int32), 0, Ww - 1)
        r1 = jnp.clip(r0 + 1, 0, Hh - 1)
        c1 = jnp.clip(c0 + 1, 0, Ww - 1)
        ar = r - r0.astype(r.dtype)
        ac = c - c0.astype(c.dtype)
        wa = (1 - ar) * (1 - ac); wb = (1 - ar) * ac
        wc = ar * (1 - ac); wd = ar * ac
        return r0, c0, r1, c1, wa, wb, wc, wd

    def bilinear_gather(img, pts):
        Hh, Ww = img.shape[0], img.shape[1]
        r0, c0, r1, c1, wa, wb, wc, wd = _corners(pts, Hh, Ww)
        if img.ndim == 3:
            wa, wb, wc, wd = wa[:, None], wb[:, None], wc[:, None], wd[:, None]
        return (img[r0, c0] * wa + img[r0, c1] * wb
                + img[r1, c0] * wc + img[r1, c1] * wd)

    def one_image_pre(pred, gt):
        gt_pts, gt_m = extract_zc(jax.lax.stop_gradient(gt))
        pr_pts, pr_m = extract_zc(jax.lax.stop_gradient(pred))
        normals = compute_normals(pred)
        n = bilinear_gather(normals, pr_pts)
        n = n / (jnp.linalg.norm(n, axis=1, keepdims=True) + 1e-8)
        gp = jnp.where(gt_m[:, None], gt_pts, BIG)
        sp = (pr_pts ** 2).sum(1)
        sg = (gp ** 2).sum(1)
        return pr_pts, pr_m, gp, n, sp, sg

    def one_image_post(pred, pr_pts, pr_m, gp, n, md2, idx):
        Hh, Ww = pred.shape
        near = gp[idx]
        contrib = pr_m & (md2 <= DIST_THRESHOLD ** 2)
        dirv = near - pr_pts
        dot = jnp.where(contrib, (dirv * n).sum(1) * UPDATE_SCALE, 0.0)
        r0, c0, r1, c1, wa, wb, wc, wd = _corners(pr_pts, Hh, Ww)
        dSDF = jnp.zeros_like(pred)
        dSDF = (dSDF.at[r0, c0].add(dot * wa)
                     .at[r0, c1].add(dot * wb)
                     .at[r1, c0].add(dot * wc)
                     .at[r1, c1].add(dot * wd))
        dSDF = jax.lax.stop_gradient(dSDF)
        inj = jnp.sum(pred * dSDF)
        vals = bilinear_gather(pred, pr_pts)
        pix = jnp.sum(jnp.where(pr_m, vals, 0.0))
        return inj, pix

    return jax, jnp, one_image_pre, one_image_post


# ---------------------------------------------------------------------------
# Host-side stripe construction
# ---------------------------------------------------------------------------

def _make_blocks():
    """Partition the 18240 pred candidates into 2D grid patches (<=128 each)
    and precompute, per block, the gt-candidate stripe (patch +-4 window in
    both grid dims, ascending global index). Input-independent."""
    blocks = []
    for base, nr, ncols in ((0, H - 1, W), (NV, H, W - 1)):   # V grid, H grid
        rstarts = list(range(0, nr, PATCH))
        cstarts = list(range(0, ncols, PATCH))
        for r0 in rstarts:
            r1 = min(r0 + PATCH, nr) - 1
            for c0 in cstarts:
                c1 = min(c0 + PATCH, ncols) - 1
                rows = np.arange(r0, r1 + 1)
                cols = np.arange(c0, c1 + 1)
                pred_ids = (base + rows[:, None] * ncols + cols[None, :]).ravel()
                nlist = []
                # gt-V window
                vr0, vr1 = max(0, r0 - 4), min(H - 2, r1 + 4)
                vc0, vc1 = max(0, c0 - 4), min(W - 1, c1 + 4)
                for r in range(vr0, vr1 + 1):
                    nlist.append(np.arange(r * W + vc0, r * W + vc1 + 1))
                # gt-H window
                hr0, hr1 = max(0, r0 - 4), min(H - 1, r1 + 4)
                hc0, hc1 = max(0, c0 - 4), min(W - 2, c1 + 4)
                for r in range(hr0, hr1 + 1):
                    nlist.append(np.arange(NV + r * (W - 1) + hc0,
                                           NV + r * (W - 1) + hc1 + 1))
                nlist = np.concatenate(nlist)
                assert len(pred_ids) <= BLK and len(nlist) <= STRIPE, \
                    (len(pred_ids), len(nlist))
                blocks.append((pred_ids, nlist))
    return blocks


_BLOCKS = _make_blocks()
NBLOCKS = len(_BLOCKS)                       # 162
BPC = (NBLOCKS + NCORES - 1) // NCORES       # 21 blocks per core (6 pad slots)


def _build_host_inputs(pr_pts, sp, gp, sg):
    """Per-core packed input arrays + pos->global-n maps.

    in1[b] = [2, 128+STRIPE]: cols 0:128 = (2*pr_r; 2*pr_c), rest = (g_r; g_c)
    nsg[b] = [1, STRIPE]: negated sg over the stripe (pads -> -1e30)
    nsp[b] = [128, 1]:    negated sp for the block's pred candidates
    """
    nslots = NCORES * BPC
    in1 = np.zeros((nslots, 2, BLK + STRIPE), np.float32)
    nsg = np.full((nslots, 1, STRIPE), np.float32(-1e30), np.float32)
    nsp = np.zeros((nslots, BLK, 1), np.float32)
    for b, (pred_ids, nlist) in enumerate(_BLOCKS):
        npred = len(pred_ids)
        L = len(nlist)
        in1[b, 0, :npred] = 2.0 * pr_pts[pred_ids, 0]
        in1[b, 1, :npred] = 2.0 * pr_pts[pred_ids, 1]
        nsp[b, :npred, 0] = -sp[pred_ids]
        in1[b, 0, BLK:BLK + L] = gp[nlist, 0]
        in1[b, 1, BLK:BLK + L] = gp[nlist, 1]
        nsg[b, 0, :L] = -sg[nlist]
    return in1, nsg, nsp


# ---------------------------------------------------------------------------
# Bass SPMD kernel
# ---------------------------------------------------------------------------

def _build_bass():
    if "nc" in _BASS_CACHE:
        return _BASS_CACHE["nc"]
    import concourse.mybir as mybir
    from concourse import bacc
    from concourse.tile import TileContext

    F32 = mybir.dt.float32
    U32 = mybir.dt.uint32
    nc = bacc.Bacc()
    i_in1 = nc.declare_dram_parameter("in1", [BPC, 2, BLK + STRIPE], F32,
                                      isOutput=False)
    i_nsg = nc.declare_dram_parameter("nsg", [BPC, 1, STRIPE], F32,
                                      isOutput=False)
    i_nsp = nc.declare_dram_parameter("nsp", [BPC, BLK, 1], F32,
                                      isOutput=False)
    o_max = nc.declare_dram_parameter("omax", [BLK, BPC * 8], F32, isOutput=True)
    o_idx = nc.declare_dram_parameter("oidx", [BLK, BPC * 8], U32, isOutput=True)

    with TileContext(nc) as tc:
        with tc.tile_pool(name="sb", bufs=4) as sb, \
             tc.tile_pool(name="res", bufs=1) as res, \
             tc.tile_pool(name="pst", bufs=3, space="PSUM") as pst:
            mxall = res.tile([BLK, BPC * 8], F32, tag="mxall")
            miall = res.tile([BLK, BPC * 8], U32, tag="miall")
            for b in range(BPC):
                a1 = sb.tile([2, BLK + STRIPE], F32, tag="a1")
                nc.gpsimd.dma_start(out=a1[:], in_=i_in1[b])
                nspcol = sb.tile([BLK, 1], F32, tag="nspcol")
                nc.gpsimd.dma_start(out=nspcol[:], in_=i_nsp[b])
                nsgb = sb.tile([BLK, STRIPE], F32, tag="nsgb")
                nc.sync.dma_start(out=nsgb[:],
                                  in_=i_nsg[b].partition_broadcast(BLK))

                p_t = pst.tile([BLK, STRIPE], F32, tag="pt")
                for q0 in range(0, STRIPE, 512):
                    q1 = min(q0 + 512, STRIPE)
                    nc.tensor.matmul(p_t[:, q0:q1], a1[:, 0:BLK],
                                     a1[:, BLK + q0:BLK + q1],
                                     start=True, stop=True)
                d2n = sb.tile([BLK, STRIPE], F32, tag="d2n")
                nc.vector.scalar_tensor_tensor(
                    out=d2n[:], in0=nsgb[:], scalar=nspcol[:, 0:1],
                    in1=p_t[:], op0=mybir.AluOpType.add,
                    op1=mybir.AluOpType.add)
                d2c = sb.tile([BLK, STRIPE], F32, tag="d2c")
                nc.vector.tensor_scalar_min(d2c[:], d2n[:], 0.0)
                nc.vector.max(out=mxall[:, b * 8:(b + 1) * 8], in_=d2c[:])
                nc.vector.max_index(out=miall[:, b * 8:(b + 1) * 8],
                                    in_max=mxall[:, b * 8:(b + 1) * 8],
                                    in_values=d2c[:])
            nc.sync.dma_start(out=o_max[:], in_=mxall[:])
            nc.sync.dma_start(out=o_idx[:], in_=miall[:])
    nc.finalize()
    _BASS_CACHE["nc"] = nc
    return nc


def _run_bass(in1, nsg, nsp, trace=False):
    from concourse.bass_utils import run_bass_kernel_spmd
    nc = _build_bass()
    core_ids = list(range(NCORES))
    in_maps = []
    for c in range(NCORES):
        sl = slice(c * BPC, (c + 1) * BPC)
        in_maps.append({"in1": np.ascontiguousarray(in1[sl]),
                        "nsg": np.ascontiguousarray(nsg[sl]),
                        "nsp": np.ascontiguousarray(nsp[sl])})
    res = run_bass_kernel_spmd(nc, in_maps, core_ids, trace=trace)
    return res


def _assemble(res):
    md2 = np.zeros(NP, np.float32)
    idx = np.zeros(NP, np.int32)
    for c in range(NCORES):
        omax = res.results[c]["omax"]          # [128, BPC*8]
        oidx = res.results[c]["oidx"]
        for bb in range(BPC):
            b = c * BPC + bb
            if b >= NBLOCKS:
                continue
            pred_ids, nlist = _BLOCKS[b]
            npred = len(pred_ids)
            mv = omax[:npred, bb * 8]
            pos = oidx[:npred, bb * 8].astype(np.int64)
            md2[pred_ids] = -mv
            idx[pred_ids] = nlist[np.minimum(pos, len(nlist) - 1)]
    return md2, idx


# ---------------------------------------------------------------------------
# Entry point
# ---------------------------------------------------------------------------

def kernel(pred_sdf, gt_sdf, _debug=None):
    jax, jnp, one_image_pre, one_image_post = _jnp_funcs()
    predb = jnp.asarray(pred_sdf)
    gtb = jnp.asarray(gt_sdf)

    pr_ptsb, pr_mb, gpb, nb, spb, sgb = jax.vmap(one_image_pre)(predb, gtb)
    pr_pts = np.asarray(pr_ptsb)[0]
    sp = np.asarray(spb)[0]
    gp = np.asarray(gpb)[0]
    sg = np.asarray(sgb)[0]

    in1, nsg, nsp = _build_host_inputs(pr_pts, sp, gp, sg)
    res = _run_bass(in1, nsg, nsp, trace=_PROFILE)
    if _PROFILE:
        global _LAST_EXEC_NS, _LAST_TRACE
        _LAST_EXEC_NS = res.exec_time_ns
        _LAST_TRACE = res
    md2, idx = _assemble(res)

    md2b = jnp.asarray(md2[None])
    idxb = jnp.asarray(idx[None])
    inj, pix = jax.vmap(one_image_post)(predb, pr_ptsb, pr_mb, gpb, nb,
                                        md2b, idxb)
    inject = inj.mean()
    pixel = pix.mean()
    out = W_INJECT * inject + W_PIXEL * pixel
    if _debug is not None:
        _debug.update(md2=md2, idx=idx, pr_pts=pr_pts, gp=gp, sp=sp, sg=sg)
    return np.asarray(out)


# revision 20
# speedup vs baseline: 1.0273x; 1.0273x over previous
"""Chamfer boundary SDF loss — Trainium2 Bass kernel (8 NeuronCores).

Strategy
--------
The reference output is a sum of f32 values interpolated exactly at SDF zero
crossings — analytically ~0, so the returned scalar is dominated by f32
rounding detail. Passing a relative-error gate therefore requires replicating
the reference's f32 arithmetic bit-exactly on the same backend (neuron/XLA
eager), not just approximating the math.

Decomposition:
  * All O(H*W) "cheap" ops (zero-crossing extraction, normals, bilinear
    weights, final gathers/scatter/sums) run as the *identical* eager jnp op
    sequence the reference executes (vmapped, B=1) — bit-identical by
    construction.
  * The O(M*N) nearest-neighbor search (M=N=18240 candidates, the dominant
    cost) runs on the 8 NeuronCores as a Bass SPMD kernel, sharded over the
    pred-candidate dim (143 blocks of 128). Each block only searches the gt
    candidates within +-4 grid rows (a "stripe" of <=2048 of the 18240
    candidates): any pred point whose true NN is farther than DIST_THRESHOLD=3
    contributes 0, and the stripe provably contains every global minimizer
    (and all argmin ties) whenever the threshold test can pass.
  * d2 bits match the reference exactly: the PE computes fl(2*cross) via a
    K=2 f32 matmul with pre-doubled pred coords (doubling commutes with
    round-to-nearest), and fl(sp+sg) via a second K=2 matmul against a ones
    row (a single-rounded fma of an exact product is an IEEE add). The DVE
    subtracts and clamps; max8/max_index implement first-occurrence argmin
    (on negated values) exactly like XLA's argmin combiner.
"""
import numpy as np

H = W = 96
NV = (H - 1) * W          # 9120 vertical-pair candidates
NH = H * (W - 1)          # 9120 horizontal-pair candidates
NP = NV + NH              # 18240
BLK = 128
NCORES = 8
PATCH = 11                # pred candidates per block = an 11x11 grid patch
STRIPE = 768              # gt candidates searched per block (patch +-4 window)
UPDATE_SCALE = 1.0
DIST_THRESHOLD = 3.0
W_INJECT = 1.0
W_PIXEL = 1.0
EPS = 1e-8
BIG = 1e6

_BASS_CACHE = {}
_PROFILE = False          # set True (e.g. by test.py) to capture a neuron profile
_LAST_EXEC_NS = None      # exec_time_ns of the last profiled run
_LAST_TRACE = None        # BassKernelResults of the last profiled run


# ---------------------------------------------------------------------------
# Eager jnp pieces — identical op sequences to the reference (vmapped, B=1)
# ---------------------------------------------------------------------------

def _jnp_funcs():
    import jax
    import jax.numpy as jnp

    def extract_zc(sdf):
        Hh, Ww = sdf.shape
        ii = jnp.arange(Hh, dtype=sdf.dtype)
        jj = jnp.arange(Ww, dtype=sdf.dtype)
        v1, v2 = sdf[:-1, :], sdf[1:, :]
        a = jnp.abs(v1) / (jnp.abs(v1) + jnp.abs(v2) + EPS)
        i0 = ii[:-1, None]
        rv = jnp.where(v1 == 0, i0, jnp.where(v2 == 0, i0 + 1.0, i0 + a))
        cv = jnp.broadcast_to(jj[None, :], v1.shape)
        mv = (v1 == 0) | (v2 == 0) | (v1 * v2 < 0)
        h1, h2 = sdf[:, :-1], sdf[:, 1:]
        b = jnp.abs(h1) / (jnp.abs(h1) + jnp.abs(h2) + EPS)
        j0 = jj[None, :-1]
        ch = jnp.where(h1 == 0, j0, jnp.where(h2 == 0, j0 + 1.0, j0 + b))
        rh = jnp.broadcast_to(ii[:, None], h1.shape)
        mh = (h1 == 0) | (h2 == 0) | (h1 * h2 < 0)
        pts = jnp.concatenate([
            jnp.stack([rv.ravel(), cv.ravel()], axis=1),
            jnp.stack([rh.ravel(), ch.ravel()], axis=1)], axis=0)
        mask = jnp.concatenate([mv.ravel(), mh.ravel()], axis=0)
        return pts, mask

    def compute_normals(sdf):
        gr = jnp.concatenate([sdf[1:2] - sdf[0:1], (sdf[2:] - sdf[:-2]) / 2.0,
                              sdf[-1:] - sdf[-2:-1]], axis=0)
        gc = jnp.concatenate([sdf[:, 1:2] - sdf[:, 0:1],
                              (sdf[:, 2:] - sdf[:, :-2]) / 2.0,
                              sdf[:, -1:] - sdf[:, -2:-1]], axis=1)
        return jnp.stack([gr, gc], axis=2)

    def _corners(pts, Hh, Ww):
        r, c = pts[:, 0], pts[:, 1]
        r0 = jnp.clip(jnp.floor(r).astype(jnp.int32), 0, Hh - 1)
        c0 = jnp.clip(jnp.floor(c).astype(jnp.int32), 0, Ww - 1)
        r1 = jnp.clip(r0 + 1, 0, Hh - 1)
        c1 = jnp.clip(c0 + 1, 0, Ww - 1)
        ar = r - r0.astype(r.dtype)
        ac = c - c0.astype(c.dtype)
        wa = (1 - ar) * (1 - ac); wb = (1 - ar) * ac
        wc = ar * (1 - ac); wd = ar * ac
        return r0, c0, r1, c1, wa, wb, wc, wd

    def bilinear_gather(img, pts):
        Hh, Ww = img.shape[0], img.shape[1]
        r0, c0, r1, c1, wa, wb, wc, wd = _corners(pts, Hh, Ww)
        if img.ndim == 3:
            wa, wb, wc, wd = wa[:, None], wb[:, None], wc[:, None], wd[:, None]
        return (img[r0, c0] * wa + img[r0, c1] * wb
                + img[r1, c0] * wc + img[r1, c1] * wd)

    def one_image_pre(pred, gt):
        gt_pts, gt_m = extract_zc(jax.lax.stop_gradient(gt))
        pr_pts, pr_m = extract_zc(jax.lax.stop_gradient(pred))
        normals = compute_normals(pred)
        n = bilinear_gather(normals, pr_pts)
        n = n / (jnp.linalg.norm(n, axis=1, keepdims=True) + 1e-8)
        gp = jnp.where(gt_m[:, None], gt_pts, BIG)
        sp = (pr_pts ** 2).sum(1)
        sg = (gp ** 2).sum(1)
        return pr_pts, pr_m, gp, n, sp, sg

    def one_image_post(pred, pr_pts, pr_m, gp, n, md2, idx):
        Hh, Ww = pred.shape
        near = gp[idx]
        contrib = pr_m & (md2 <= DIST_THRESHOLD ** 2)
        dirv = near - pr_pts
        dot = jnp.where(contrib, (dirv * n).sum(1) * UPDATE_SCALE, 0.0)
        r0, c0, r1, c1, wa, wb, wc, wd = _corners(pr_pts, Hh, Ww)
        dSDF = jnp.zeros_like(pred)
        dSDF = (dSDF.at[r0, c0].add(dot * wa)
                     .at[r0, c1].add(dot * wb)
                     .at[r1, c0].add(dot * wc)
                     .at[r1, c1].add(dot * wd))
        dSDF = jax.lax.stop_gradient(dSDF)
        inj = jnp.sum(pred * dSDF)
        vals = bilinear_gather(pred, pr_pts)
        pix = jnp.sum(jnp.where(pr_m, vals, 0.0))
        return inj, pix

    return jax, jnp, one_image_pre, one_image_post


# ---------------------------------------------------------------------------
# Host-side stripe construction
# ---------------------------------------------------------------------------

def _make_blocks():
    """Partition the 18240 pred candidates into 2D grid patches (<=128 each)
    and precompute, per block, the gt-candidate stripe (patch +-4 window in
    both grid dims, ascending global index). Input-independent."""
    blocks = []
    for base, nr, ncols in ((0, H - 1, W), (NV, H, W - 1)):   # V grid, H grid
        rstarts = list(range(0, nr, PATCH))
        cstarts = list(range(0, ncols, PATCH))
        for r0 in rstarts:
            r1 = min(r0 + PATCH, nr) - 1
            for c0 in cstarts:
                c1 = min(c0 + PATCH, ncols) - 1
                rows = np.arange(r0, r1 + 1)
                cols = np.arange(c0, c1 + 1)
                pred_ids = (base + rows[:, None] * ncols + cols[None, :]).ravel()
                nlist = []
                # gt-V window
                vr0, vr1 = max(0, r0 - 4), min(H - 2, r1 + 4)
                vc0, vc1 = max(0, c0 - 4), min(W - 1, c1 + 4)
                for r in range(vr0, vr1 + 1):
                    nlist.append(np.arange(r * W + vc0, r * W + vc1 + 1))
                # gt-H window
                hr0, hr1 = max(0, r0 - 4), min(H - 1, r1 + 4)
                hc0, hc1 = max(0, c0 - 4), min(W - 2, c1 + 4)
                for r in range(hr0, hr1 + 1):
                    nlist.append(np.arange(NV + r * (W - 1) + hc0,
                                           NV + r * (W - 1) + hc1 + 1))
                nlist = np.concatenate(nlist)
                assert len(pred_ids) <= BLK and len(nlist) <= STRIPE, \
                    (len(pred_ids), len(nlist))
                blocks.append((pred_ids, nlist))
    return blocks


_BLOCKS = _make_blocks()
NBLOCKS = len(_BLOCKS)                       # 162
BPC = (NBLOCKS + NCORES - 1) // NCORES       # 21 blocks per core (6 pad slots)


def _build_host_inputs(pr_pts, sp, gp, sg):
    """Per-core packed input arrays + pos->global-n maps.

    in1[b] = [2, 128+STRIPE]: cols 0:128 = (2*pr_r; 2*pr_c), rest = (g_r; g_c)
    nsg[b] = [1, STRIPE]: negated sg over the stripe (pads -> -1e30)
    nsp[b] = [128, 1]:    negated sp for the block's pred candidates
    """
    nslots = NCORES * BPC
    in1 = np.zeros((nslots, 2, BLK + STRIPE), np.float32)
    nsg = np.full((nslots, 1, STRIPE), np.float32(-1e30), np.float32)
    nsp = np.zeros((nslots, BLK, 1), np.float32)
    for b, (pred_ids, nlist) in enumerate(_BLOCKS):
        npred = len(pred_ids)
        L = len(nlist)
        in1[b, 0, :npred] = 2.0 * pr_pts[pred_ids, 0]
        in1[b, 1, :npred] = 2.0 * pr_pts[pred_ids, 1]
        nsp[b, :npred, 0] = -sp[pred_ids]
        in1[b, 0, BLK:BLK + L] = gp[nlist, 0]
        in1[b, 1, BLK:BLK + L] = gp[nlist, 1]
        nsg[b, 0, :L] = -sg[nlist]
    return in1, nsg, nsp


# ---------------------------------------------------------------------------
# Bass SPMD kernel
# ---------------------------------------------------------------------------

def _build_bass():
    if "nc" in _BASS_CACHE:
        return _BASS_CACHE["nc"]
    import concourse.mybir as mybir
    from concourse import bacc
    from concourse.tile import TileContext

    F32 = mybir.dt.float32
    U32 = mybir.dt.uint32
    nc = bacc.Bacc()
    i_in1 = nc.declare_dram_parameter("in1", [BPC, 2, BLK + STRIPE], F32,
                                      isOutput=False)
    i_nsg = nc.declare_dram_parameter("nsg", [BPC, 1, STRIPE], F32,
                                      isOutput=False)
    i_nsp = nc.declare_dram_parameter("nsp", [BPC, BLK, 1], F32,
                                      isOutput=False)
    o_max = nc.declare_dram_parameter("omax", [BLK, BPC * 8], F32, isOutput=True)
    o_idx = nc.declare_dram_parameter("oidx", [BLK, BPC * 8], U32, isOutput=True)

    with TileContext(nc) as tc:
        with tc.tile_pool(name="sb", bufs=4) as sb, \
             tc.tile_pool(name="res", bufs=1) as res, \
             tc.tile_pool(name="pst", bufs=3, space="PSUM") as pst:
            mxall = res.tile([BLK, BPC * 8], F32, tag="mxall")
            miall = res.tile([BLK, BPC * 8], U32, tag="miall")
            for b in range(BPC):
                a1 = sb.tile([2, BLK + STRIPE], F32, tag="a1")
                nc.sync.dma_start(out=a1[:], in_=i_in1[b])
                nspcol = sb.tile([BLK, 1], F32, tag="nspcol")
                nc.sync.dma_start(out=nspcol[:], in_=i_nsp[b])
                nsgb = sb.tile([BLK, STRIPE], F32, tag="nsgb")
                nc.sync.dma_start(out=nsgb[:],
                                  in_=i_nsg[b].partition_broadcast(BLK))

                p_t = pst.tile([BLK, STRIPE], F32, tag="pt")
                for q0 in range(0, STRIPE, 512):
                    q1 = min(q0 + 512, STRIPE)
                    nc.tensor.matmul(p_t[:, q0:q1], a1[:, 0:BLK],
                                     a1[:, BLK + q0:BLK + q1],
                                     start=True, stop=True)
                d2n = sb.tile([BLK, STRIPE], F32, tag="d2n")
                nc.vector.scalar_tensor_tensor(
                    out=d2n[:], in0=nsgb[:], scalar=nspcol[:, 0:1],
                    in1=p_t[:], op0=mybir.AluOpType.add,
                    op1=mybir.AluOpType.add)
                d2c = sb.tile([BLK, STRIPE], F32, tag="d2c")
                nc.vector.tensor_scalar_min(d2c[:], d2n[:], 0.0)
                nc.vector.max(out=mxall[:, b * 8:(b + 1) * 8], in_=d2c[:])
                nc.vector.max_index(out=miall[:, b * 8:(b + 1) * 8],
                                    in_max=mxall[:, b * 8:(b + 1) * 8],
                                    in_values=d2c[:])
            nc.sync.dma_start(out=o_max[:], in_=mxall[:])
            nc.sync.dma_start(out=o_idx[:], in_=miall[:])
    nc.finalize()
    _BASS_CACHE["nc"] = nc
    return nc


def _run_bass(in1, nsg, nsp, trace=False):
    from concourse.bass_utils import run_bass_kernel_spmd
    nc = _build_bass()
    core_ids = list(range(NCORES))
    in_maps = []
    for c in range(NCORES):
        sl = slice(c * BPC, (c + 1) * BPC)
        in_maps.append({"in1": np.ascontiguousarray(in1[sl]),
                        "nsg": np.ascontiguousarray(nsg[sl]),
                        "nsp": np.ascontiguousarray(nsp[sl])})
    res = run_bass_kernel_spmd(nc, in_maps, core_ids, trace=trace)
    return res


def _assemble(res):
    md2 = np.zeros(NP, np.float32)
    idx = np.zeros(NP, np.int32)
    for c in range(NCORES):
        omax = res.results[c]["omax"]          # [128, BPC*8]
        oidx = res.results[c]["oidx"]
        for bb in range(BPC):
            b = c * BPC + bb
            if b >= NBLOCKS:
                continue
            pred_ids, nlist = _BLOCKS[b]
            npred = len(pred_ids)
            mv = omax[:npred, bb * 8]
            pos = oidx[:npred, bb * 8].astype(np.int64)
            md2[pred_ids] = -mv
            idx[pred_ids] = nlist[np.minimum(pos, len(nlist) - 1)]
    return md2, idx


# ---------------------------------------------------------------------------
# Entry point
# ---------------------------------------------------------------------------

def kernel(pred_sdf, gt_sdf, _debug=None):
    jax, jnp, one_image_pre, one_image_post = _jnp_funcs()
    predb = jnp.asarray(pred_sdf)
    gtb = jnp.asarray(gt_sdf)

    pr_ptsb, pr_mb, gpb, nb, spb, sgb = jax.vmap(one_image_pre)(predb, gtb)
    pr_pts = np.asarray(pr_ptsb)[0]
    sp = np.asarray(spb)[0]
    gp = np.asarray(gpb)[0]
    sg = np.asarray(sgb)[0]

    in1, nsg, nsp = _build_host_inputs(pr_pts, sp, gp, sg)
    res = _run_bass(in1, nsg, nsp, trace=_PROFILE)
    if _PROFILE:
        global _LAST_EXEC_NS, _LAST_TRACE
        _LAST_EXEC_NS = res.exec_time_ns
        _LAST_TRACE = res
    md2, idx = _assemble(res)

    md2b = jnp.asarray(md2[None])
    idxb = jnp.asarray(idx[None])
    inj, pix = jax.vmap(one_image_post)(predb, pr_ptsb, pr_mb, gpb, nb,
                                        md2b, idxb)
    inject = inj.mean()
    pixel = pix.mean()
    out = W_INJECT * inject + W_PIXEL * pixel
    if _debug is not None:
        _debug.update(md2=md2, idx=idx, pr_pts=pr_pts, gp=gp, sp=sp, sg=sg)
    return np.asarray(out)


# revision 23
# speedup vs baseline: 1.2286x; 1.1960x over previous
"""Chamfer boundary SDF loss — Trainium2 Bass kernel (8 NeuronCores).

Strategy
--------
The reference output is a sum of f32 values interpolated exactly at SDF zero
crossings — analytically ~0, so the returned scalar is dominated by f32
rounding detail. Passing a relative-error gate therefore requires replicating
the reference's f32 arithmetic bit-exactly on the same backend (neuron/XLA
eager), not just approximating the math.

Decomposition:
  * All O(H*W) "cheap" ops (zero-crossing extraction, normals, bilinear
    weights, final gathers/scatter/sums) run as the *identical* eager jnp op
    sequence the reference executes (vmapped, B=1) — bit-identical by
    construction.
  * The O(M*N) nearest-neighbor search (M=N=18240 candidates, the dominant
    cost) runs on the 8 NeuronCores as a Bass SPMD kernel, sharded over the
    pred-candidate dim (143 blocks of 128). Each block only searches the gt
    candidates within +-4 grid rows (a "stripe" of <=2048 of the 18240
    candidates): any pred point whose true NN is farther than DIST_THRESHOLD=3
    contributes 0, and the stripe provably contains every global minimizer
    (and all argmin ties) whenever the threshold test can pass.
  * d2 bits match the reference exactly: the PE computes fl(2*cross) via a
    K=2 f32 matmul with pre-doubled pred coords (doubling commutes with
    round-to-nearest), and fl(sp+sg) via a second K=2 matmul against a ones
    row (a single-rounded fma of an exact product is an IEEE add). The DVE
    subtracts and clamps; max8/max_index implement first-occurrence argmin
    (on negated values) exactly like XLA's argmin combiner.
"""
import numpy as np

H = W = 96
NV = (H - 1) * W          # 9120 vertical-pair candidates
NH = H * (W - 1)          # 9120 horizontal-pair candidates
NP = NV + NH              # 18240
BLK = 128
NCORES = 8
PATCH_R = 16              # pred candidates per block = a 16x8 grid patch
PATCH_C = 8
STRIPE = 704              # gt candidates searched per block (patch window)
UPDATE_SCALE = 1.0
DIST_THRESHOLD = 3.0
W_INJECT = 1.0
W_PIXEL = 1.0
EPS = 1e-8
BIG = 1e6

_BASS_CACHE = {}
_PROFILE = False          # set True (e.g. by test.py) to capture a neuron profile
_LAST_EXEC_NS = None      # exec_time_ns of the last profiled run
_LAST_TRACE = None        # BassKernelResults of the last profiled run


# ---------------------------------------------------------------------------
# Eager jnp pieces — identical op sequences to the reference (vmapped, B=1)
# ---------------------------------------------------------------------------

def _jnp_funcs():
    import jax
    import jax.numpy as jnp

    def extract_zc(sdf):
        Hh, Ww = sdf.shape
        ii = jnp.arange(Hh, dtype=sdf.dtype)
        jj = jnp.arange(Ww, dtype=sdf.dtype)
        v1, v2 = sdf[:-1, :], sdf[1:, :]
        a = jnp.abs(v1) / (jnp.abs(v1) + jnp.abs(v2) + EPS)
        i0 = ii[:-1, None]
        rv = jnp.where(v1 == 0, i0, jnp.where(v2 == 0, i0 + 1.0, i0 + a))
        cv = jnp.broadcast_to(jj[None, :], v1.shape)
        mv = (v1 == 0) | (v2 == 0) | (v1 * v2 < 0)
        h1, h2 = sdf[:, :-1], sdf[:, 1:]
        b = jnp.abs(h1) / (jnp.abs(h1) + jnp.abs(h2) + EPS)
        j0 = jj[None, :-1]
        ch = jnp.where(h1 == 0, j0, jnp.where(h2 == 0, j0 + 1.0, j0 + b))
        rh = jnp.broadcast_to(ii[:, None], h1.shape)
        mh = (h1 == 0) | (h2 == 0) | (h1 * h2 < 0)
        pts = jnp.concatenate([
            jnp.stack([rv.ravel(), cv.ravel()], axis=1),
            jnp.stack([rh.ravel(), ch.ravel()], axis=1)], axis=0)
        mask = jnp.concatenate([mv.ravel(), mh.ravel()], axis=0)
        return pts, mask

    def compute_normals(sdf):
        gr = jnp.concatenate([sdf[1:2] - sdf[0:1], (sdf[2:] - sdf[:-2]) / 2.0,
                              sdf[-1:] - sdf[-2:-1]], axis=0)
        gc = jnp.concatenate([sdf[:, 1:2] - sdf[:, 0:1],
                              (sdf[:, 2:] - sdf[:, :-2]) / 2.0,
                              sdf[:, -1:] - sdf[:, -2:-1]], axis=1)
        return jnp.stack([gr, gc], axis=2)

    def _corners(pts, Hh, Ww):
        r, c = pts[:, 0], pts[:, 1]
        r0 = jnp.clip(jnp.floor(r).astype(jnp.int32), 0, Hh - 1)
        c0 = jnp.clip(jnp.floor(c).astype(jnp.int32), 0, Ww - 1)
        r1 = jnp.clip(r0 + 1, 0, Hh - 1)
        c1 = jnp.clip(c0 + 1, 0, Ww - 1)
        ar = r - r0.astype(r.dtype)
        ac = c - c0.astype(c.dtype)
        wa = (1 - ar) * (1 - ac); wb = (1 - ar) * ac
        wc = ar * (1 - ac); wd = ar * ac
        return r0, c0, r1, c1, wa, wb, wc, wd

    def bilinear_gather(img, pts):
        Hh, Ww = img.shape[0], img.shape[1]
        r0, c0, r1, c1, wa, wb, wc, wd = _corners(pts, Hh, Ww)
        if img.ndim == 3:
            wa, wb, wc, wd = wa[:, None], wb[:, None], wc[:, None], wd[:, None]
        return (img[r0, c0] * wa + img[r0, c1] * wb
                + img[r1, c0] * wc + img[r1, c1] * wd)

    def one_image_pre(pred, gt):
        gt_pts, gt_m = extract_zc(jax.lax.stop_gradient(gt))
        pr_pts, pr_m = extract_zc(jax.lax.stop_gradient(pred))
        normals = compute_normals(pred)
        n = bilinear_gather(normals, pr_pts)
        n = n / (jnp.linalg.norm(n, axis=1, keepdims=True) + 1e-8)
        gp = jnp.where(gt_m[:, None], gt_pts, BIG)
        sp = (pr_pts ** 2).sum(1)
        sg = (gp ** 2).sum(1)
        return pr_pts, pr_m, gp, n, sp, sg

    def one_image_post(pred, pr_pts, pr_m, gp, n, md2, idx):
        Hh, Ww = pred.shape
        near = gp[idx]
        contrib = pr_m & (md2 <= DIST_THRESHOLD ** 2)
        dirv = near - pr_pts
        dot = jnp.where(contrib, (dirv * n).sum(1) * UPDATE_SCALE, 0.0)
        r0, c0, r1, c1, wa, wb, wc, wd = _corners(pr_pts, Hh, Ww)
        dSDF = jnp.zeros_like(pred)
        dSDF = (dSDF.at[r0, c0].add(dot * wa)
                     .at[r0, c1].add(dot * wb)
                     .at[r1, c0].add(dot * wc)
                     .at[r1, c1].add(dot * wd))
        dSDF = jax.lax.stop_gradient(dSDF)
        inj = jnp.sum(pred * dSDF)
        vals = bilinear_gather(pred, pr_pts)
        pix = jnp.sum(jnp.where(pr_m, vals, 0.0))
        return inj, pix

    return jax, jnp, one_image_pre, one_image_post


# ---------------------------------------------------------------------------
# Host-side stripe construction
# ---------------------------------------------------------------------------

def _make_blocks():
    """Partition the 18240 pred candidates into 16x8 grid patches (=128 each)
    and precompute, per block, the gt-candidate stripe: the tight per-type
    window that provably contains every gt candidate within distance
    DIST_THRESHOLD of any pred candidate in the patch (and hence the global
    argmin plus all its ties whenever the threshold can pass). Segments are
    emitted in ascending global index so stripe-position order reproduces
    jnp.argmin's first-occurrence tie-break. Input-independent.

    Window bounds (pred type vs gt type, grid row/col offsets relative to the
    patch span): V/V rows +-4, cols +-3; V/H rows [-3,+4], cols [-4,+3];
    H/V rows [-4,+3], cols [-3,+4]; H/H rows +-3, cols +-4.
    """
    blocks = []
    for fam, (base, nr, ncols) in enumerate(((0, H - 1, W), (NV, H, W - 1))):
        if fam == 0:           # V preds
            vwin = (4, 4, 3, 3)    # gt-V: rows -4..+4, cols -3..+3
            hwin = (3, 4, 4, 3)    # gt-H: rows -3..+4, cols -4..+3
        else:                  # H preds
            vwin = (4, 3, 3, 4)
            hwin = (3, 3, 4, 4)
        for r0 in range(0, nr, PATCH_R):
            r1 = min(r0 + PATCH_R, nr) - 1
            for c0 in range(0, ncols, PATCH_C):
                c1 = min(c0 + PATCH_C, ncols) - 1
                rows = np.arange(r0, r1 + 1)
                cols = np.arange(c0, c1 + 1)
                pred_ids = (base + rows[:, None] * ncols + cols[None, :]).ravel()
                nlist = []
                ru, rd, cu, cd = vwin
                vr0, vr1 = max(0, r0 - ru), min(H - 2, r1 + rd)
                vc0, vc1 = max(0, c0 - cu), min(W - 1, c1 + cd)
                for r in range(vr0, vr1 + 1):
                    nlist.append(np.arange(r * W + vc0, r * W + vc1 + 1))
                ru, rd, cu, cd = hwin
                hr0, hr1 = max(0, r0 - ru), min(H - 1, r1 + rd)
                hc0, hc1 = max(0, c0 - cu), min(W - 2, c1 + cd)
                for r in range(hr0, hr1 + 1):
                    nlist.append(np.arange(NV + r * (W - 1) + hc0,
                                           NV + r * (W - 1) + hc1 + 1))
                nlist = np.concatenate(nlist)
                assert len(pred_ids) <= BLK and len(nlist) <= STRIPE, \
                    (len(pred_ids), len(nlist))
                blocks.append((pred_ids, nlist))
    return blocks


_BLOCKS = _make_blocks()
NBLOCKS = len(_BLOCKS)                       # 162
BPC = (NBLOCKS + NCORES - 1) // NCORES       # 21 blocks per core (6 pad slots)


def _build_host_inputs(pr_pts, sp, gp, sg):
    """Per-core packed input arrays + pos->global-n maps.

    in1[b] = [2, 128+STRIPE]: cols 0:128 = (2*pr_r; 2*pr_c), rest = (g_r; g_c)
    nsg[b] = [1, STRIPE]: negated sg over the stripe (pads -> -1e30)
    nsp[b] = [128, 1]:    negated sp for the block's pred candidates
    """
    nslots = NCORES * BPC
    in1 = np.zeros((nslots, 2, BLK + STRIPE), np.float32)
    nsg = np.full((nslots, 1, STRIPE), np.float32(-1e30), np.float32)
    nsp = np.zeros((nslots, BLK, 1), np.float32)
    for b, (pred_ids, nlist) in enumerate(_BLOCKS):
        npred = len(pred_ids)
        L = len(nlist)
        in1[b, 0, :npred] = 2.0 * pr_pts[pred_ids, 0]
        in1[b, 1, :npred] = 2.0 * pr_pts[pred_ids, 1]
        nsp[b, :npred, 0] = -sp[pred_ids]
        in1[b, 0, BLK:BLK + L] = gp[nlist, 0]
        in1[b, 1, BLK:BLK + L] = gp[nlist, 1]
        nsg[b, 0, :L] = -sg[nlist]
    return in1, nsg, nsp


# ---------------------------------------------------------------------------
# Bass SPMD kernel
# ---------------------------------------------------------------------------

def _build_bass():
    if "nc" in _BASS_CACHE:
        return _BASS_CACHE["nc"]
    import concourse.mybir as mybir
    from concourse import bacc
    from concourse.tile import TileContext

    F32 = mybir.dt.float32
    U32 = mybir.dt.uint32
    nc = bacc.Bacc()
    i_in1 = nc.declare_dram_parameter("in1", [BPC, 2, BLK + STRIPE], F32,
                                      isOutput=False)
    i_nsg = nc.declare_dram_parameter("nsg", [BPC, 1, STRIPE], F32,
                                      isOutput=False)
    i_nsp = nc.declare_dram_parameter("nsp", [BPC, BLK, 1], F32,
                                      isOutput=False)
    o_max = nc.declare_dram_parameter("omax", [BLK, BPC * 8], F32, isOutput=True)
    o_idx = nc.declare_dram_parameter("oidx", [BLK, BPC * 8], U32, isOutput=True)

    with TileContext(nc) as tc:
        with tc.tile_pool(name="sb", bufs=4) as sb, \
             tc.tile_pool(name="res", bufs=1) as res, \
             tc.tile_pool(name="pst", bufs=3, space="PSUM") as pst:
            mxall = res.tile([BLK, BPC * 8], F32, tag="mxall")
            miall = res.tile([BLK, BPC * 8], U32, tag="miall")
            for b in range(BPC):
                a1 = sb.tile([2, BLK + STRIPE], F32, tag="a1")
                nc.sync.dma_start(out=a1[:], in_=i_in1[b])
                nspcol = sb.tile([BLK, 1], F32, tag="nspcol")
                nc.sync.dma_start(out=nspcol[:], in_=i_nsp[b])
                nsgb = sb.tile([BLK, STRIPE], F32, tag="nsgb")
                nc.sync.dma_start(out=nsgb[:],
                                  in_=i_nsg[b].partition_broadcast(BLK))

                p_t = pst.tile([BLK, STRIPE], F32, tag="pt")
                for q0 in range(0, STRIPE, 512):
                    q1 = min(q0 + 512, STRIPE)
                    nc.tensor.matmul(p_t[:, q0:q1], a1[:, 0:BLK],
                                     a1[:, BLK + q0:BLK + q1],
                                     start=True, stop=True)
                t1s = sb.tile([BLK, STRIPE], F32, tag="t1s")
                nc.scalar.activation(t1s[:], p_t[:],
                                     mybir.ActivationFunctionType.Copy)
                d2n = sb.tile([BLK, STRIPE], F32, tag="d2n")
                nc.vector.scalar_tensor_tensor(
                    out=d2n[:], in0=nsgb[:], scalar=nspcol[:, 0:1],
                    in1=t1s[:], op0=mybir.AluOpType.add,
                    op1=mybir.AluOpType.add)
                d2c = sb.tile([BLK, STRIPE], F32, tag="d2c")
                nc.vector.tensor_scalar_min(d2c[:], d2n[:], 0.0)
                nc.vector.max(out=mxall[:, b * 8:(b + 1) * 8], in_=d2c[:])
                nc.vector.max_index(out=miall[:, b * 8:(b + 1) * 8],
                                    in_max=mxall[:, b * 8:(b + 1) * 8],
                                    in_values=d2c[:])
            nc.sync.dma_start(out=o_max[:], in_=mxall[:])
            nc.sync.dma_start(out=o_idx[:], in_=miall[:])
    nc.finalize()
    _BASS_CACHE["nc"] = nc
    return nc


def _run_bass(in1, nsg, nsp, trace=False):
    from concourse.bass_utils import run_bass_kernel_spmd
    nc = _build_bass()
    core_ids = list(range(NCORES))
    in_maps = []
    for c in range(NCORES):
        sl = slice(c * BPC, (c + 1) * BPC)
        in_maps.append({"in1": np.ascontiguousarray(in1[sl]),
                        "nsg": np.ascontiguousarray(nsg[sl]),
                        "nsp": np.ascontiguousarray(nsp[sl])})
    res = run_bass_kernel_spmd(nc, in_maps, core_ids, trace=trace)
    return res


def _assemble(res):
    md2 = np.zeros(NP, np.float32)
    idx = np.zeros(NP, np.int32)
    for c in range(NCORES):
        omax = res.results[c]["omax"]          # [128, BPC*8]
        oidx = res.results[c]["oidx"]
        for bb in range(BPC):
            b = c * BPC + bb
            if b >= NBLOCKS:
                continue
            pred_ids, nlist = _BLOCKS[b]
            npred = len(pred_ids)
            mv = omax[:npred, bb * 8]
            pos = oidx[:npred, bb * 8].astype(np.int64)
            md2[pred_ids] = -mv
            idx[pred_ids] = nlist[np.minimum(pos, len(nlist) - 1)]
    return md2, idx


# ---------------------------------------------------------------------------
# Entry point
# ---------------------------------------------------------------------------

def kernel(pred_sdf, gt_sdf, _debug=None):
    jax, jnp, one_image_pre, one_image_post = _jnp_funcs()
    predb = jnp.asarray(pred_sdf)
    gtb = jnp.asarray(gt_sdf)

    pr_ptsb, pr_mb, gpb, nb, spb, sgb = jax.vmap(one_image_pre)(predb, gtb)
    pr_pts = np.asarray(pr_ptsb)[0]
    sp = np.asarray(spb)[0]
    gp = np.asarray(gpb)[0]
    sg = np.asarray(sgb)[0]

    in1, nsg, nsp = _build_host_inputs(pr_pts, sp, gp, sg)
    res = _run_bass(in1, nsg, nsp, trace=_PROFILE)
    if _PROFILE:
        global _LAST_EXEC_NS, _LAST_TRACE
        _LAST_EXEC_NS = res.exec_time_ns
        _LAST_TRACE = res
    md2, idx = _assemble(res)

    md2b = jnp.asarray(md2[None])
    idxb = jnp.asarray(idx[None])
    inj, pix = jax.vmap(one_image_post)(predb, pr_ptsb, pr_mb, gpb, nb,
                                        md2b, idxb)
    inject = inj.mean()
    pixel = pix.mean()
    out = W_INJECT * inject + W_PIXEL * pixel
    if _debug is not None:
        _debug.update(md2=md2, idx=idx, pr_pts=pr_pts, gp=gp, sp=sp, sg=sg)
    return np.asarray(out)


# revision 25
# speedup vs baseline: 1.2395x; 1.0089x over previous
"""Chamfer boundary SDF loss — Trainium2 Bass kernel (8 NeuronCores).

Strategy
--------
The reference output is a sum of f32 values interpolated exactly at SDF zero
crossings — analytically ~0, so the returned scalar is dominated by f32
rounding detail. Passing a relative-error gate therefore requires replicating
the reference's f32 arithmetic bit-exactly on the same backend (neuron/XLA
eager), not just approximating the math.

Decomposition:
  * All O(H*W) "cheap" ops (zero-crossing extraction, normals, bilinear
    weights, final gathers/scatter/sums) run as the *identical* eager jnp op
    sequence the reference executes (vmapped, B=1) — bit-identical by
    construction.
  * The O(M*N) nearest-neighbor search (M=N=18240 candidates, the dominant
    cost) runs on the 8 NeuronCores as a Bass SPMD kernel, sharded over the
    pred-candidate dim (143 blocks of 128). Each block only searches the gt
    candidates within +-4 grid rows (a "stripe" of <=2048 of the 18240
    candidates): any pred point whose true NN is farther than DIST_THRESHOLD=3
    contributes 0, and the stripe provably contains every global minimizer
    (and all argmin ties) whenever the threshold test can pass.
  * d2 bits match the reference exactly: the PE computes fl(2*cross) via a
    K=2 f32 matmul with pre-doubled pred coords (doubling commutes with
    round-to-nearest), and fl(sp+sg) via a second K=2 matmul against a ones
    row (a single-rounded fma of an exact product is an IEEE add). The DVE
    subtracts and clamps; max8/max_index implement first-occurrence argmin
    (on negated values) exactly like XLA's argmin combiner.
"""
import numpy as np

H = W = 96
NV = (H - 1) * W          # 9120 vertical-pair candidates
NH = H * (W - 1)          # 9120 horizontal-pair candidates
NP = NV + NH              # 18240
BLK = 128
NCORES = 8
PATCH_R = 16              # pred candidates per block = a 16x8 grid patch
PATCH_C = 8
STRIPE = 704              # gt candidates searched per block (patch window)
UPDATE_SCALE = 1.0
DIST_THRESHOLD = 3.0
W_INJECT = 1.0
W_PIXEL = 1.0
EPS = 1e-8
BIG = 1e6

_BASS_CACHE = {}
_PROFILE = False          # set True (e.g. by test.py) to capture a neuron profile
_LAST_EXEC_NS = None      # exec_time_ns of the last profiled run
_LAST_TRACE = None        # BassKernelResults of the last profiled run


# ---------------------------------------------------------------------------
# Eager jnp pieces — identical op sequences to the reference (vmapped, B=1)
# ---------------------------------------------------------------------------

def _jnp_funcs():
    import jax
    import jax.numpy as jnp

    def extract_zc(sdf):
        Hh, Ww = sdf.shape
        ii = jnp.arange(Hh, dtype=sdf.dtype)
        jj = jnp.arange(Ww, dtype=sdf.dtype)
        v1, v2 = sdf[:-1, :], sdf[1:, :]
        a = jnp.abs(v1) / (jnp.abs(v1) + jnp.abs(v2) + EPS)
        i0 = ii[:-1, None]
        rv = jnp.where(v1 == 0, i0, jnp.where(v2 == 0, i0 + 1.0, i0 + a))
        cv = jnp.broadcast_to(jj[None, :], v1.shape)
        mv = (v1 == 0) | (v2 == 0) | (v1 * v2 < 0)
        h1, h2 = sdf[:, :-1], sdf[:, 1:]
        b = jnp.abs(h1) / (jnp.abs(h1) + jnp.abs(h2) + EPS)
        j0 = jj[None, :-1]
        ch = jnp.where(h1 == 0, j0, jnp.where(h2 == 0, j0 + 1.0, j0 + b))
        rh = jnp.broadcast_to(ii[:, None], h1.shape)
        mh = (h1 == 0) | (h2 == 0) | (h1 * h2 < 0)
        pts = jnp.concatenate([
            jnp.stack([rv.ravel(), cv.ravel()], axis=1),
            jnp.stack([rh.ravel(), ch.ravel()], axis=1)], axis=0)
        mask = jnp.concatenate([mv.ravel(), mh.ravel()], axis=0)
        return pts, mask

    def compute_normals(sdf):
        gr = jnp.concatenate([sdf[1:2] - sdf[0:1], (sdf[2:] - sdf[:-2]) / 2.0,
                              sdf[-1:] - sdf[-2:-1]], axis=0)
        gc = jnp.concatenate([sdf[:, 1:2] - sdf[:, 0:1],
                              (sdf[:, 2:] - sdf[:, :-2]) / 2.0,
                              sdf[:, -1:] - sdf[:, -2:-1]], axis=1)
        return jnp.stack([gr, gc], axis=2)

    def _corners(pts, Hh, Ww):
        r, c = pts[:, 0], pts[:, 1]
        r0 = jnp.clip(jnp.floor(r).astype(jnp.int32), 0, Hh - 1)
        c0 = jnp.clip(jnp.floor(c).astype(jnp.int32), 0, Ww - 1)
        r1 = jnp.clip(r0 + 1, 0, Hh - 1)
        c1 = jnp.clip(c0 + 1, 0, Ww - 1)
        ar = r - r0.astype(r.dtype)
        ac = c - c0.astype(c.dtype)
        wa = (1 - ar) * (1 - ac); wb = (1 - ar) * ac
        wc = ar * (1 - ac); wd = ar * ac
        return r0, c0, r1, c1, wa, wb, wc, wd

    def bilinear_gather(img, pts):
        Hh, Ww = img.shape[0], img.shape[1]
        r0, c0, r1, c1, wa, wb, wc, wd = _corners(pts, Hh, Ww)
        if img.ndim == 3:
            wa, wb, wc, wd = wa[:, None], wb[:, None], wc[:, None], wd[:, None]
        return (img[r0, c0] * wa + img[r0, c1] * wb
                + img[r1, c0] * wc + img[r1, c1] * wd)

    def one_image_pre(pred, gt):
        gt_pts, gt_m = extract_zc(jax.lax.stop_gradient(gt))
        pr_pts, pr_m = extract_zc(jax.lax.stop_gradient(pred))
        normals = compute_normals(pred)
        n = bilinear_gather(normals, pr_pts)
        n = n / (jnp.linalg.norm(n, axis=1, keepdims=True) + 1e-8)
        gp = jnp.where(gt_m[:, None], gt_pts, BIG)
        sp = (pr_pts ** 2).sum(1)
        sg = (gp ** 2).sum(1)
        return pr_pts, pr_m, gp, n, sp, sg

    def one_image_post(pred, pr_pts, pr_m, gp, n, md2, idx):
        Hh, Ww = pred.shape
        near = gp[idx]
        contrib = pr_m & (md2 <= DIST_THRESHOLD ** 2)
        dirv = near - pr_pts
        dot = jnp.where(contrib, (dirv * n).sum(1) * UPDATE_SCALE, 0.0)
        r0, c0, r1, c1, wa, wb, wc, wd = _corners(pr_pts, Hh, Ww)
        dSDF = jnp.zeros_like(pred)
        dSDF = (dSDF.at[r0, c0].add(dot * wa)
                     .at[r0, c1].add(dot * wb)
                     .at[r1, c0].add(dot * wc)
                     .at[r1, c1].add(dot * wd))
        dSDF = jax.lax.stop_gradient(dSDF)
        inj = jnp.sum(pred * dSDF)
        vals = bilinear_gather(pred, pr_pts)
        pix = jnp.sum(jnp.where(pr_m, vals, 0.0))
        return inj, pix

    return jax, jnp, one_image_pre, one_image_post


# ---------------------------------------------------------------------------
# Host-side stripe construction
# ---------------------------------------------------------------------------

def _make_blocks():
    """Partition the 18240 pred candidates into 16x8 grid patches (=128 each)
    and precompute, per block, the gt-candidate stripe: the tight per-type
    window that provably contains every gt candidate within distance
    DIST_THRESHOLD of any pred candidate in the patch (and hence the global
    argmin plus all its ties whenever the threshold can pass). Segments are
    emitted in ascending global index so stripe-position order reproduces
    jnp.argmin's first-occurrence tie-break. Input-independent.

    Window bounds (pred type vs gt type, grid row/col offsets relative to the
    patch span): V/V rows +-4, cols +-3; V/H rows [-3,+4], cols [-4,+3];
    H/V rows [-4,+3], cols [-3,+4]; H/H rows +-3, cols +-4.
    """
    blocks = []
    for fam, (base, nr, ncols) in enumerate(((0, H - 1, W), (NV, H, W - 1))):
        if fam == 0:           # V preds
            vwin = (4, 4, 3, 3)    # gt-V: rows -4..+4, cols -3..+3
            hwin = (3, 4, 4, 3)    # gt-H: rows -3..+4, cols -4..+3
        else:                  # H preds
            vwin = (4, 3, 3, 4)
            hwin = (3, 3, 4, 4)
        for r0 in range(0, nr, PATCH_R):
            r1 = min(r0 + PATCH_R, nr) - 1
            for c0 in range(0, ncols, PATCH_C):
                c1 = min(c0 + PATCH_C, ncols) - 1
                rows = np.arange(r0, r1 + 1)
                cols = np.arange(c0, c1 + 1)
                pred_ids = (base + rows[:, None] * ncols + cols[None, :]).ravel()
                nlist = []
                ru, rd, cu, cd = vwin
                vr0, vr1 = max(0, r0 - ru), min(H - 2, r1 + rd)
                vc0, vc1 = max(0, c0 - cu), min(W - 1, c1 + cd)
                for r in range(vr0, vr1 + 1):
                    nlist.append(np.arange(r * W + vc0, r * W + vc1 + 1))
                ru, rd, cu, cd = hwin
                hr0, hr1 = max(0, r0 - ru), min(H - 1, r1 + rd)
                hc0, hc1 = max(0, c0 - cu), min(W - 2, c1 + cd)
                for r in range(hr0, hr1 + 1):
                    nlist.append(np.arange(NV + r * (W - 1) + hc0,
                                           NV + r * (W - 1) + hc1 + 1))
                nlist = np.concatenate(nlist)
                assert len(pred_ids) <= BLK and len(nlist) <= STRIPE, \
                    (len(pred_ids), len(nlist))
                blocks.append((pred_ids, nlist))
    return blocks


_BLOCKS = _make_blocks()
NBLOCKS = len(_BLOCKS)                       # 162
BPC = (NBLOCKS + NCORES - 1) // NCORES       # 21 blocks per core (6 pad slots)


def _build_host_inputs(pr_pts, sp, gp, sg):
    """Per-core packed input arrays + pos->global-n maps.

    in1[b] = [2, 128+STRIPE]: cols 0:128 = (2*pr_r; 2*pr_c), rest = (g_r; g_c)
    nsg[b] = [1, STRIPE]: negated sg over the stripe (pads -> -1e30)
    nsp[b] = [128, 1]:    negated sp for the block's pred candidates
    """
    nslots = NCORES * BPC
    in1 = np.zeros((nslots, 2, BLK + STRIPE), np.float32)
    nsg = np.full((nslots, 1, STRIPE), np.float32(-1e30), np.float32)
    nsp = np.zeros((nslots, BLK, 1), np.float32)
    for b, (pred_ids, nlist) in enumerate(_BLOCKS):
        npred = len(pred_ids)
        L = len(nlist)
        in1[b, 0, :npred] = 2.0 * pr_pts[pred_ids, 0]
        in1[b, 1, :npred] = 2.0 * pr_pts[pred_ids, 1]
        nsp[b, :npred, 0] = -sp[pred_ids]
        in1[b, 0, BLK:BLK + L] = gp[nlist, 0]
        in1[b, 1, BLK:BLK + L] = gp[nlist, 1]
        nsg[b, 0, :L] = -sg[nlist]
    return in1, nsg, nsp


# ---------------------------------------------------------------------------
# Bass SPMD kernel
# ---------------------------------------------------------------------------

def _build_bass():
    if "nc" in _BASS_CACHE:
        return _BASS_CACHE["nc"]
    import concourse.mybir as mybir
    from concourse import bacc
    from concourse.tile import TileContext

    F32 = mybir.dt.float32
    U32 = mybir.dt.uint32
    nc = bacc.Bacc()
    i_in1 = nc.declare_dram_parameter("in1", [BPC, 2, BLK + STRIPE], F32,
                                      isOutput=False)
    i_nsg = nc.declare_dram_parameter("nsg", [BPC, 1, STRIPE], F32,
                                      isOutput=False)
    i_nsp = nc.declare_dram_parameter("nsp", [BPC, BLK, 1], F32,
                                      isOutput=False)
    o_max = nc.declare_dram_parameter("omax", [BLK, BPC * 8], F32, isOutput=True)
    o_idx = nc.declare_dram_parameter("oidx", [BLK, BPC * 8], U32, isOutput=True)

    with TileContext(nc) as tc:
        with tc.tile_pool(name="sb", bufs=6) as sb, \
             tc.tile_pool(name="res", bufs=1) as res, \
             tc.tile_pool(name="pst", bufs=4, space="PSUM") as pst:
            mxall = res.tile([BLK, BPC * 8], F32, tag="mxall")
            miall = res.tile([BLK, BPC * 8], U32, tag="miall")
            for b in range(BPC):
                a1 = sb.tile([2, BLK + STRIPE], F32, tag="a1")
                nc.sync.dma_start(out=a1[:], in_=i_in1[b])
                nspcol = sb.tile([BLK, 1], F32, tag="nspcol")
                nc.sync.dma_start(out=nspcol[:], in_=i_nsp[b])
                nsgb = sb.tile([BLK, STRIPE], F32, tag="nsgb")
                nc.sync.dma_start(out=nsgb[:],
                                  in_=i_nsg[b].partition_broadcast(BLK))

                p_t = pst.tile([BLK, STRIPE], F32, tag="pt")
                for q0 in range(0, STRIPE, 512):
                    q1 = min(q0 + 512, STRIPE)
                    nc.tensor.matmul(p_t[:, q0:q1], a1[:, 0:BLK],
                                     a1[:, BLK + q0:BLK + q1],
                                     start=True, stop=True)
                d2n = sb.tile([BLK, STRIPE], F32, tag="d2n")
                nc.vector.scalar_tensor_tensor(
                    out=d2n[:], in0=nsgb[:], scalar=nspcol[:, 0:1],
                    in1=p_t[:], op0=mybir.AluOpType.add,
                    op1=mybir.AluOpType.add)
                d2c = sb.tile([BLK, STRIPE], F32, tag="d2c")
                nc.vector.tensor_scalar_min(d2c[:], d2n[:], 0.0)
                nc.vector.max(out=mxall[:, b * 8:(b + 1) * 8], in_=d2c[:])
                nc.vector.max_index(out=miall[:, b * 8:(b + 1) * 8],
                                    in_max=mxall[:, b * 8:(b + 1) * 8],
                                    in_values=d2c[:])
            nc.sync.dma_start(out=o_max[:], in_=mxall[:])
            nc.sync.dma_start(out=o_idx[:], in_=miall[:])
    nc.finalize()
    _BASS_CACHE["nc"] = nc
    return nc


def _run_bass(in1, nsg, nsp, trace=False):
    from concourse.bass_utils import run_bass_kernel_spmd
    nc = _build_bass()
    core_ids = list(range(NCORES))
    in_maps = []
    for c in range(NCORES):
        sl = slice(c * BPC, (c + 1) * BPC)
        in_maps.append({"in1": np.ascontiguousarray(in1[sl]),
                        "nsg": np.ascontiguousarray(nsg[sl]),
                        "nsp": np.ascontiguousarray(nsp[sl])})
    res = run_bass_kernel_spmd(nc, in_maps, core_ids, trace=trace)
    return res


def _assemble(res):
    md2 = np.zeros(NP, np.float32)
    idx = np.zeros(NP, np.int32)
    for c in range(NCORES):
        omax = res.results[c]["omax"]          # [128, BPC*8]
        oidx = res.results[c]["oidx"]
        for bb in range(BPC):
            b = c * BPC + bb
            if b >= NBLOCKS:
                continue
            pred_ids, nlist = _BLOCKS[b]
            npred = len(pred_ids)
            mv = omax[:npred, bb * 8]
            pos = oidx[:npred, bb * 8].astype(np.int64)
            md2[pred_ids] = -mv
            idx[pred_ids] = nlist[np.minimum(pos, len(nlist) - 1)]
    return md2, idx


# ---------------------------------------------------------------------------
# Entry point
# ---------------------------------------------------------------------------

def kernel(pred_sdf, gt_sdf, _debug=None):
    jax, jnp, one_image_pre, one_image_post = _jnp_funcs()
    predb = jnp.asarray(pred_sdf)
    gtb = jnp.asarray(gt_sdf)

    pr_ptsb, pr_mb, gpb, nb, spb, sgb = jax.vmap(one_image_pre)(predb, gtb)
    pr_pts = np.asarray(pr_ptsb)[0]
    sp = np.asarray(spb)[0]
    gp = np.asarray(gpb)[0]
    sg = np.asarray(sgb)[0]

    in1, nsg, nsp = _build_host_inputs(pr_pts, sp, gp, sg)
    res = _run_bass(in1, nsg, nsp, trace=_PROFILE)
    if _PROFILE:
        global _LAST_EXEC_NS, _LAST_TRACE
        _LAST_EXEC_NS = res.exec_time_ns
        _LAST_TRACE = res
    md2, idx = _assemble(res)

    md2b = jnp.asarray(md2[None])
    idxb = jnp.asarray(idx[None])
    inj, pix = jax.vmap(one_image_post)(predb, pr_ptsb, pr_mb, gpb, nb,
                                        md2b, idxb)
    inject = inj.mean()
    pixel = pix.mean()
    out = W_INJECT * inject + W_PIXEL * pixel
    if _debug is not None:
        _debug.update(md2=md2, idx=idx, pr_pts=pr_pts, gp=gp, sp=sp, sg=sg)
    return np.asarray(out)


# revision 29
# speedup vs baseline: 1.3205x; 1.0653x over previous
"""Chamfer boundary SDF loss — Trainium2 Bass kernel (8 NeuronCores).

Strategy
--------
The reference output is a sum of f32 values interpolated exactly at SDF zero
crossings — analytically ~0, so the returned scalar is dominated by f32
rounding detail. Passing a relative-error gate therefore requires replicating
the reference's f32 arithmetic bit-exactly on the same backend (neuron/XLA
eager), not just approximating the math.

Decomposition:
  * All O(H*W) "cheap" ops (zero-crossing extraction, normals, bilinear
    weights, final gathers/scatter/sums) run as the *identical* eager jnp op
    sequence the reference executes (vmapped, B=1) — bit-identical by
    construction.
  * The O(M*N) nearest-neighbor search (M=N=18240 candidates, the dominant
    cost) runs on the 8 NeuronCores as a Bass SPMD kernel, sharded over the
    pred-candidate dim (143 blocks of 128). Each block only searches the gt
    candidates within +-4 grid rows (a "stripe" of <=2048 of the 18240
    candidates): any pred point whose true NN is farther than DIST_THRESHOLD=3
    contributes 0, and the stripe provably contains every global minimizer
    (and all argmin ties) whenever the threshold test can pass.
  * d2 bits match the reference exactly: the PE computes fl(2*cross) via a
    K=2 f32 matmul with pre-doubled pred coords (doubling commutes with
    round-to-nearest), and fl(sp+sg) via a second K=2 matmul against a ones
    row (a single-rounded fma of an exact product is an IEEE add). The DVE
    subtracts and clamps; max8/max_index implement first-occurrence argmin
    (on negated values) exactly like XLA's argmin combiner.
"""
import numpy as np

H = W = 96
NV = (H - 1) * W          # 9120 vertical-pair candidates
NH = H * (W - 1)          # 9120 horizontal-pair candidates
NP = NV + NH              # 18240
BLK = 128
NCORES = 8
PATCH_R = 16              # pred candidates per block = a 16x8 grid patch
PATCH_C = 8
STRIPE = 704              # gt candidates searched per block (patch window)
UPDATE_SCALE = 1.0
DIST_THRESHOLD = 3.0
W_INJECT = 1.0
W_PIXEL = 1.0
EPS = 1e-8
BIG = 1e6

_BASS_CACHE = {}
_PROFILE = False          # set True (e.g. by test.py) to capture a neuron profile
_LAST_EXEC_NS = None      # exec_time_ns of the last profiled run
_LAST_TRACE = None        # BassKernelResults of the last profiled run


# ---------------------------------------------------------------------------
# Eager jnp pieces — identical op sequences to the reference (vmapped, B=1)
# ---------------------------------------------------------------------------

def _jnp_funcs():
    import jax
    import jax.numpy as jnp

    def extract_zc(sdf):
        Hh, Ww = sdf.shape
        ii = jnp.arange(Hh, dtype=sdf.dtype)
        jj = jnp.arange(Ww, dtype=sdf.dtype)
        v1, v2 = sdf[:-1, :], sdf[1:, :]
        a = jnp.abs(v1) / (jnp.abs(v1) + jnp.abs(v2) + EPS)
        i0 = ii[:-1, None]
        rv = jnp.where(v1 == 0, i0, jnp.where(v2 == 0, i0 + 1.0, i0 + a))
        cv = jnp.broadcast_to(jj[None, :], v1.shape)
        mv = (v1 == 0) | (v2 == 0) | (v1 * v2 < 0)
        h1, h2 = sdf[:, :-1], sdf[:, 1:]
        b = jnp.abs(h1) / (jnp.abs(h1) + jnp.abs(h2) + EPS)
        j0 = jj[None, :-1]
        ch = jnp.where(h1 == 0, j0, jnp.where(h2 == 0, j0 + 1.0, j0 + b))
        rh = jnp.broadcast_to(ii[:, None], h1.shape)
        mh = (h1 == 0) | (h2 == 0) | (h1 * h2 < 0)
        pts = jnp.concatenate([
            jnp.stack([rv.ravel(), cv.ravel()], axis=1),
            jnp.stack([rh.ravel(), ch.ravel()], axis=1)], axis=0)
        mask = jnp.concatenate([mv.ravel(), mh.ravel()], axis=0)
        return pts, mask

    def compute_normals(sdf):
        gr = jnp.concatenate([sdf[1:2] - sdf[0:1], (sdf[2:] - sdf[:-2]) / 2.0,
                              sdf[-1:] - sdf[-2:-1]], axis=0)
        gc = jnp.concatenate([sdf[:, 1:2] - sdf[:, 0:1],
                              (sdf[:, 2:] - sdf[:, :-2]) / 2.0,
                              sdf[:, -1:] - sdf[:, -2:-1]], axis=1)
        return jnp.stack([gr, gc], axis=2)

    def _corners(pts, Hh, Ww):
        r, c = pts[:, 0], pts[:, 1]
        r0 = jnp.clip(jnp.floor(r).astype(jnp.int32), 0, Hh - 1)
        c0 = jnp.clip(jnp.floor(c).astype(jnp.int32), 0, Ww - 1)
        r1 = jnp.clip(r0 + 1, 0, Hh - 1)
        c1 = jnp.clip(c0 + 1, 0, Ww - 1)
        ar = r - r0.astype(r.dtype)
        ac = c - c0.astype(c.dtype)
        wa = (1 - ar) * (1 - ac); wb = (1 - ar) * ac
        wc = ar * (1 - ac); wd = ar * ac
        return r0, c0, r1, c1, wa, wb, wc, wd

    def bilinear_gather(img, pts):
        Hh, Ww = img.shape[0], img.shape[1]
        r0, c0, r1, c1, wa, wb, wc, wd = _corners(pts, Hh, Ww)
        if img.ndim == 3:
            wa, wb, wc, wd = wa[:, None], wb[:, None], wc[:, None], wd[:, None]
        return (img[r0, c0] * wa + img[r0, c1] * wb
                + img[r1, c0] * wc + img[r1, c1] * wd)

    def one_image_pre(pred, gt):
        gt_pts, gt_m = extract_zc(jax.lax.stop_gradient(gt))
        pr_pts, pr_m = extract_zc(jax.lax.stop_gradient(pred))
        normals = compute_normals(pred)
        n = bilinear_gather(normals, pr_pts)
        n = n / (jnp.linalg.norm(n, axis=1, keepdims=True) + 1e-8)
        gp = jnp.where(gt_m[:, None], gt_pts, BIG)
        sp = (pr_pts ** 2).sum(1)
        sg = (gp ** 2).sum(1)
        return pr_pts, pr_m, gp, n, sp, sg

    def one_image_post(pred, pr_pts, pr_m, gp, n, md2, idx):
        Hh, Ww = pred.shape
        near = gp[idx]
        contrib = pr_m & (md2 <= DIST_THRESHOLD ** 2)
        dirv = near - pr_pts
        dot = jnp.where(contrib, (dirv * n).sum(1) * UPDATE_SCALE, 0.0)
        r0, c0, r1, c1, wa, wb, wc, wd = _corners(pr_pts, Hh, Ww)
        dSDF = jnp.zeros_like(pred)
        dSDF = (dSDF.at[r0, c0].add(dot * wa)
                     .at[r0, c1].add(dot * wb)
                     .at[r1, c0].add(dot * wc)
                     .at[r1, c1].add(dot * wd))
        dSDF = jax.lax.stop_gradient(dSDF)
        inj = jnp.sum(pred * dSDF)
        vals = bilinear_gather(pred, pr_pts)
        pix = jnp.sum(jnp.where(pr_m, vals, 0.0))
        return inj, pix

    return jax, jnp, one_image_pre, one_image_post


# ---------------------------------------------------------------------------
# Host-side stripe construction
# ---------------------------------------------------------------------------

def _make_blocks():
    """Partition the 18240 pred candidates into 16x8 grid patches (=128 each)
    and precompute, per block, the gt-candidate stripe: the tight per-type
    window that provably contains every gt candidate within distance
    DIST_THRESHOLD of any pred candidate in the patch (and hence the global
    argmin plus all its ties whenever the threshold can pass). Segments are
    emitted in ascending global index so stripe-position order reproduces
    jnp.argmin's first-occurrence tie-break. Input-independent.

    Window bounds (pred type vs gt type, grid row/col offsets relative to the
    patch span): V/V rows +-4, cols +-3; V/H rows [-3,+4], cols [-4,+3];
    H/V rows [-4,+3], cols [-3,+4]; H/H rows +-3, cols +-4.
    """
    blocks = []
    for fam, (base, nr, ncols) in enumerate(((0, H - 1, W), (NV, H, W - 1))):
        if fam == 0:           # V preds
            vwin = (4, 4, 3, 3)    # gt-V: rows -4..+4, cols -3..+3
            hwin = (3, 4, 4, 3)    # gt-H: rows -3..+4, cols -4..+3
        else:                  # H preds
            vwin = (4, 3, 3, 4)
            hwin = (3, 3, 4, 4)
        for r0 in range(0, nr, PATCH_R):
            r1 = min(r0 + PATCH_R, nr) - 1
            for c0 in range(0, ncols, PATCH_C):
                c1 = min(c0 + PATCH_C, ncols) - 1
                rows = np.arange(r0, r1 + 1)
                cols = np.arange(c0, c1 + 1)
                pred_ids = (base + rows[:, None] * ncols + cols[None, :]).ravel()
                nlist = []
                ru, rd, cu, cd = vwin
                vr0, vr1 = max(0, r0 - ru), min(H - 2, r1 + rd)
                vc0, vc1 = max(0, c0 - cu), min(W - 1, c1 + cd)
                for r in range(vr0, vr1 + 1):
                    nlist.append(np.arange(r * W + vc0, r * W + vc1 + 1))
                ru, rd, cu, cd = hwin
                hr0, hr1 = max(0, r0 - ru), min(H - 1, r1 + rd)
                hc0, hc1 = max(0, c0 - cu), min(W - 2, c1 + cd)
                for r in range(hr0, hr1 + 1):
                    nlist.append(np.arange(NV + r * (W - 1) + hc0,
                                           NV + r * (W - 1) + hc1 + 1))
                nlist = np.concatenate(nlist)
                assert len(pred_ids) <= BLK and len(nlist) <= STRIPE, \
                    (len(pred_ids), len(nlist))
                blocks.append((pred_ids, nlist))
    return blocks


_BLOCKS = _make_blocks()
NBLOCKS = len(_BLOCKS)                       # 162
BPC = (NBLOCKS + NCORES - 1) // NCORES       # 21 blocks per core (6 pad slots)


def _build_host_inputs(pr_pts, sp, gp, sg):
    """Per-core packed input arrays + pos->global-n maps.

    in1[b] = [2, 128+STRIPE]: cols 0:128 = (2*pr_r; 2*pr_c), rest = (g_r; g_c)
    nsg[b] = [1, STRIPE]: negated sg over the stripe (pads -> -1e30)
    nsp[b] = [128, 1]:    negated sp for the block's pred candidates
    """
    nslots = NCORES * BPC
    in1 = np.zeros((nslots, 2, BLK + STRIPE), np.float32)
    nsg = np.full((nslots, 1, STRIPE), np.float32(-1e30), np.float32)
    nsp = np.zeros((nslots, BLK, 1), np.float32)
    for b, (pred_ids, nlist) in enumerate(_BLOCKS):
        npred = len(pred_ids)
        L = len(nlist)
        in1[b, 0, :npred] = 2.0 * pr_pts[pred_ids, 0]
        in1[b, 1, :npred] = 2.0 * pr_pts[pred_ids, 1]
        nsp[b, :npred, 0] = -sp[pred_ids]
        in1[b, 0, BLK:BLK + L] = gp[nlist, 0]
        in1[b, 1, BLK:BLK + L] = gp[nlist, 1]
        nsg[b, 0, :L] = -sg[nlist]
    return in1, nsg, nsp


# ---------------------------------------------------------------------------
# Bass SPMD kernel
# ---------------------------------------------------------------------------

def _build_bass():
    if "nc" in _BASS_CACHE:
        return _BASS_CACHE["nc"]
    import concourse.mybir as mybir
    from concourse import bacc
    from concourse.tile import TileContext

    F32 = mybir.dt.float32
    U32 = mybir.dt.uint32
    nc = bacc.Bacc()
    i_in1 = nc.declare_dram_parameter("in1", [BPC, 2, BLK + STRIPE], F32,
                                      isOutput=False)
    i_nsg = nc.declare_dram_parameter("nsg", [BPC, 1, STRIPE], F32,
                                      isOutput=False)
    i_nsp = nc.declare_dram_parameter("nsp", [BPC, BLK, 1], F32,
                                      isOutput=False)
    o_max = nc.declare_dram_parameter("omax", [BLK, BPC * 8], F32, isOutput=True)
    o_idx = nc.declare_dram_parameter("oidx", [BLK, BPC * 8], U32, isOutput=True)

    with TileContext(nc) as tc:
        with tc.tile_pool(name="sb", bufs=6) as sb, \
             tc.tile_pool(name="sbg", bufs=10) as sbg, \
             tc.tile_pool(name="res", bufs=1) as res, \
             tc.tile_pool(name="pst", bufs=4, space="PSUM") as pst:
            mxall = res.tile([BLK, BPC * 8], F32, tag="mxall")
            miall = res.tile([BLK, BPC * 8], U32, tag="miall")
            for b in range(BPC):
                a1 = sb.tile([2, BLK + STRIPE], F32, tag="a1")
                nc.sync.dma_start(out=a1[:], in_=i_in1[b])
                nspcol = sb.tile([BLK, 1], F32, tag="nspcol")
                nc.sync.dma_start(out=nspcol[:], in_=i_nsp[b])
                nsgb = sbg.tile([BLK, STRIPE], F32, tag="nsgb")
                nc.sync.dma_start(out=nsgb[:],
                                  in_=i_nsg[b].partition_broadcast(BLK))

                p_t = pst.tile([BLK, STRIPE], F32, tag="pt")
                for q0 in range(0, STRIPE, 512):
                    q1 = min(q0 + 512, STRIPE)
                    nc.tensor.matmul(p_t[:, q0:q1], a1[:, 0:BLK],
                                     a1[:, BLK + q0:BLK + q1],
                                     start=True, stop=True)
                d2n = sb.tile([BLK, STRIPE], F32, tag="d2n")
                nc.vector.scalar_tensor_tensor(
                    out=d2n[:], in0=nsgb[:], scalar=nspcol[:, 0:1],
                    in1=p_t[:], op0=mybir.AluOpType.add,
                    op1=mybir.AluOpType.add)
                # No clamp: d2n = -d2_raw (unclamped). The reference clamps
                # d2 at 0 before argmin; since at most 4 gt candidates can
                # lie within the cancellation radius of one pred point
                # (<=2 per edge orientation), every "clamp-tied" candidate
                # (d2n >= 0) is guaranteed to appear in the top-8, and the
                # host reconstructs the clamped first-occurrence argmin
                # from the 8 (value, index) pairs exactly.
                nc.vector.max(out=mxall[:, b * 8:(b + 1) * 8], in_=d2n[:])
                nc.vector.max_index(out=miall[:, b * 8:(b + 1) * 8],
                                    in_max=mxall[:, b * 8:(b + 1) * 8],
                                    in_values=d2n[:])
            nc.sync.dma_start(out=o_max[:], in_=mxall[:])
            nc.sync.dma_start(out=o_idx[:], in_=miall[:])
    nc.finalize()
    _BASS_CACHE["nc"] = nc
    return nc


def _run_bass(in1, nsg, nsp, trace=False):
    from concourse.bass_utils import run_bass_kernel_spmd
    nc = _build_bass()
    core_ids = list(range(NCORES))
    in_maps = []
    for c in range(NCORES):
        sl = slice(c * BPC, (c + 1) * BPC)
        in_maps.append({"in1": np.ascontiguousarray(in1[sl]),
                        "nsg": np.ascontiguousarray(nsg[sl]),
                        "nsp": np.ascontiguousarray(nsp[sl])})
    res = run_bass_kernel_spmd(nc, in_maps, core_ids, trace=trace)
    return res


def _assemble(res):
    md2 = np.zeros(NP, np.float32)
    idx = np.zeros(NP, np.int32)
    for c in range(NCORES):
        omax = res.results[c]["omax"]          # [128, BPC*8]
        oidx = res.results[c]["oidx"]
        for bb in range(BPC):
            b = c * BPC + bb
            if b >= NBLOCKS:
                continue
            pred_ids, nlist = _BLOCKS[b]
            npred = len(pred_ids)
            mv8 = omax[:npred, bb * 8:(bb + 1) * 8]        # descending values
            mi8 = oidx[:npred, bb * 8:(bb + 1) * 8].astype(np.int64)
            has_nn = mv8[:, 0] >= 0      # some d2_raw <= 0 -> clamped min is 0
            md2v = np.where(has_nn, np.float32(0.0), -mv8[:, 0])
            nonneg = mv8 >= 0
            pos_nn = np.where(nonneg, mi8, 1 << 30).min(axis=1)
            pos = np.where(has_nn, pos_nn, mi8[:, 0])
            md2[pred_ids] = md2v
            idx[pred_ids] = nlist[np.minimum(pos, len(nlist) - 1)]
    return md2, idx


# ---------------------------------------------------------------------------
# Entry point
# ---------------------------------------------------------------------------

def kernel(pred_sdf, gt_sdf, _debug=None):
    jax, jnp, one_image_pre, one_image_post = _jnp_funcs()
    predb = jnp.asarray(pred_sdf)
    gtb = jnp.asarray(gt_sdf)

    pr_ptsb, pr_mb, gpb, nb, spb, sgb = jax.vmap(one_image_pre)(predb, gtb)
    pr_pts = np.asarray(pr_ptsb)[0]
    sp = np.asarray(spb)[0]
    gp = np.asarray(gpb)[0]
    sg = np.asarray(sgb)[0]

    in1, nsg, nsp = _build_host_inputs(pr_pts, sp, gp, sg)
    res = _run_bass(in1, nsg, nsp, trace=_PROFILE)
    if _PROFILE:
        global _LAST_EXEC_NS, _LAST_TRACE
        _LAST_EXEC_NS = res.exec_time_ns
        _LAST_TRACE = res
    md2, idx = _assemble(res)

    md2b = jnp.asarray(md2[None])
    idxb = jnp.asarray(idx[None])
    inj, pix = jax.vmap(one_image_post)(predb, pr_ptsb, pr_mb, gpb, nb,
                                        md2b, idxb)
    inject = inj.mean()
    pixel = pix.mean()
    out = W_INJECT * inject + W_PIXEL * pixel
    if _debug is not None:
        _debug.update(md2=md2, idx=idx, pr_pts=pr_pts, gp=gp, sp=sp, sg=sg)
    return np.asarray(out)
